# revision 1
# baseline (speedup 1.0000x reference)
"""Distributed real-vector SHT on 8 Trainium2 NeuronCores.

Full inputs in, full output out. Internally: azimuthal-mode (m) model
parallelism — each of the 8 cores computes 46 of the 368 (padded from 361)
azimuthal modes end-to-end:

  stage 1 (DFT):      y[m, r] = sum_n dft[n, m] * x[n, r]      (matmul)
  stage 2 (transp):   y_T[lat, (comp,trig,ch), m]              (PE transpose)
  stage 3 (Legendre): out[(comp,trig,ch), l] = sum_lat y_T * w (matmul per m)
  stage 4 (combine):  complex recombination of the 4 planes    (DVE adds)

All tensor math happens on-device; the host only does layout shuffles,
dtype casts and the final complex packing.
"""

import sys
import numpy as np
from contextlib import ExitStack

sys.path.insert(0, "/opt/trn_rl_repo")

import concourse.bass as bass  # noqa: E402
import concourse.tile as tile  # noqa: E402
from concourse import bacc  # noqa: E402
from concourse import mybir  # noqa: E402
from concourse.bass_utils import run_bass_kernel_spmd  # noqa: E402

NLAT, NLON = 360, 720
LMAX, MMAX = 360, 361
NCORES = 8
MPC = 46           # modes per core (8*46 = 368 >= 361, padded with zeros)
M2 = 2 * MPC       # 92: cos block + sin block
PADM = 128         # DFT output partitions: cos at 0:46, sin at 64:110
KC = 120           # contraction chunk (partitions)
NKC = 6            # longitude chunks: 6*120 = 720
LKC = 3            # latitude chunks:  3*120 = 360
CH = 32
R = 2 * CH * NLAT  # 23040 rows = (comp, ch, lat)
WBUF = 18          # weight pool slots (top-level, disjoint SBUF region)
WPRE = 18          # weights prefetched during stage 1
RT = 720           # row tile = 2 (comp,ch) pairs of one component
NRT = R // RT      # 32
NPR = RT // NLAT   # 2 pairs per row tile
F16 = mybir.dt.float16
F32 = mybir.dt.float32

_CACHE = {}


def _build_program(reps=1, mode="full"):
    nc = bacc.Bacc("TRN2", target_bir_lowering=False, debug=False,
                   num_devices=NCORES)
    xt = nc.dram_tensor("xt", [KC, NKC, R], F16, kind="ExternalInput").ap()
    dftm = nc.dram_tensor("dftm", [KC, NKC, PADM], F16, kind="ExternalInput").ap()
    gmat = nc.dram_tensor("gmat", [PADM, 2, PADM], F16, kind="ExternalInput").ap()
    wts = nc.dram_tensor("wts", [MPC, KC, LKC, 2 * LMAX], F16,
                         kind="ExternalInput").ap()
    out = nc.dram_tensor("out", [MPC, 64, 2, LMAX], F16,
                         kind="ExternalOutput").ap()

    with tile.TileContext(nc) as tc, ExitStack() as ctx:
        const_pool = ctx.enter_context(tc.tile_pool(name="const", bufs=1))
        yt_pool = ctx.enter_context(tc.tile_pool(name="yt", bufs=1))
        # weight pool lives at top level in a disjoint SBUF region so its
        # DMAs never wait on stage-1 tile frees; prefetch during stage 1.
        w_pool = ctx.enter_context(tc.tile_pool(name="win", bufs=WBUF))

        df_t = const_pool.tile([KC, NKC, PADM], F16, tag="df")
        nc.gpsimd.dma_start(df_t[:], dftm)
        g_t = const_pool.tile([PADM, 2, PADM], F16, tag="g")
        nc.gpsimd.dma_start(g_t[:], gmat)

        # y_T[kc]: [lat_part, j, (comp, trig, ch)]
        yt_t = [yt_pool.tile([KC, MPC, 2, 2, CH], F16, tag=f"yt{kc}",
                             name=f"yt{kc}")
                for kc in range(LKC)]

        if mode == "nodve":
            for kc in range(LKC):
                nc.gpsimd.memset(yt_t[kc][:], 0.0)
        for _rep in range(reps):
            _build_body(nc, tc, xt, dftm, gmat, wts, out, df_t, g_t, yt_t,
                        w_pool, mode)

    nc.compile()
    return nc


def _build_body(nc, tc, xt, dftm, gmat, wts, out, df_t, g_t, yt_t,
                w_pool, mode="full"):
    dma_only = (mode == "dma")
    no_dve = (mode in ("dma", "nodve"))
    w_tiles = {}
    if True:
        # ---- stage 1+2: DFT + transpose ----
        with tc.tile_pool(name="xin", bufs=6) as xin_pool, \
             tc.tile_pool(name="dps", bufs=2, space="PSUM") as dps_pool, \
             tc.tile_pool(name="ycp", bufs=8) as yc_pool, \
             tc.tile_pool(name="tps", bufs=4, space="PSUM") as tps_pool:
            for t in range(NRT):
                comp = t // (NRT // 2)
                x_t = xin_pool.tile([KC, NKC, RT], F16, tag="xin")
                nc.sync.dma_start(x_t[:], xt[:, :, t * RT:(t + 1) * RT])
                # prefetch stage-3 weights while stage 1 streams x
                if t < WPRE:
                    w_tiles[t] = w_pool.tile([KC, LKC, 2 * LMAX], F16,
                                             tag="win", name=f"wpre{t}")
                    nc.scalar.dma_start(w_tiles[t][:], wts[t])

                ps = [dps_pool.tile([PADM, 384], F32, tag=f"dps{h}",
                                    name=f"ps{h}") for h in range(2)]
                for h in range(2 if not dma_only else 0):
                    for kc in range(NKC):
                        nc.tensor.matmul(
                            ps[h][:, 0:360],
                            lhsT=df_t[:, kc, :],
                            rhs=x_t[:, kc, h * 360:(h + 1) * 360],
                            start=(kc == 0), stop=(kc == NKC - 1),
                        )

                yc = yc_pool.tile([PADM, RT], F16, tag="yc")
                ycv = yc.rearrange("p (a b) -> p a b", a=2, b=360)
                if no_dve:
                    if not dma_only:
                        nc.gpsimd.memset(yc[:], 0.0)
                elif comp == 0:
                    # negate sin rows so block values are (y0r, y0i);
                    # ACT takes cos halves, DVE the sin halves (PSUM-legal)
                    for h in range(2):
                        nc.scalar.copy(ycv[0:64, h], ps[h][0:64, 0:360])
                        nc.vector.tensor_scalar_mul(ycv[64:PADM, h],
                                                    ps[h][64:PADM, 0:360],
                                                    -1.0)
                else:
                    for h in range(2):
                        nc.scalar.copy(ycv[0:64, h], ps[h][0:64, 0:360])
                        nc.vector.tensor_copy(ycv[64:PADM, h],
                                              ps[h][64:PADM, 0:360])

                for c in range(NPR if not dma_only else 0):
                    ch = (t % (NRT // 2)) * NPR + c
                    for kc in range(LKC):
                        tp = tps_pool.tile([KC, 2, 64], F16, tag="tps")
                        nc.tensor.transpose(
                            tp[:],
                            yc[:, c * NLAT + kc * KC:c * NLAT + (kc + 1) * KC],
                            g_t[:, comp, :],
                        )
                        if not no_dve:
                            # scatter -> yt[kc][:, j, comp, trig, ch]
                            dst = yt_t[kc][:, :, comp, :, ch].transpose([0, 2, 1])
                            if kc == 1:
                                nc.scalar.copy(dst, tp[:, :, 0:MPC])
                            else:
                                nc.vector.tensor_copy(dst, tp[:, :, 0:MPC])

        # ---- stage 3+4: Legendre + combine ----
        with tc.tile_pool(name="lps", bufs=4, space="PSUM") as lps_pool, \
             tc.tile_pool(name="osb", bufs=8) as o_pool:
            for j in range(MPC):
                if j in w_tiles:
                    w_t = w_tiles.pop(j)
                else:
                    w_t = w_pool.tile([KC, LKC, 2 * LMAX], F16, tag="win")
                    nc.scalar.dma_start(w_t[:], wts[j])

                lp = [lps_pool.tile([128, 512], F32, tag=f"lps{h}",
                                    name=f"lp{h}_{j}") for h in range(2)]
                for h in range(2 if not dma_only else 0):
                    for kc in range(LKC):
                        nc.tensor.matmul(
                            lp[h][:, 0:LMAX],
                            lhsT=yt_t[kc][:, j],
                            rhs=w_t[:, kc, h * LMAX:(h + 1) * LMAX],
                            start=(kc == 0), stop=(kc == LKC - 1),
                        )

                osb = o_pool.tile([64, 2, LMAX], F16, tag="osb")
                if not no_dve:
                    # DVE TensorTensor may read only one PSUM operand: stage
                    # the comp-1 half through SBUF on the idle ScalarE.
                    csb = o_pool.tile([64, 2, LMAX], F32, tag="csb")
                    nc.scalar.copy(csb[:, 0, :], lp[0][64:128, 0:LMAX])
                    nc.vector.tensor_copy(csb[:, 1, :], lp[1][64:128, 0:LMAX])
                    # rows: [out0re; out0im]
                    nc.vector.tensor_add(osb[:, 0, :], lp[0][0:64, 0:LMAX],
                                         csb[:, 1, :])
                    # rows: [out1im; -out1re]; sign of out1re fixed on host
                    nc.vector.tensor_add(osb[:, 1, :], lp[1][0:64, 0:LMAX],
                                         csb[:, 0, :])
                else:
                    nc.gpsimd.memset(osb[:], 0.0)

                nc.sync.dma_start(out[j], osb[:])


def _prep_in_maps(x, weights):
    x = np.asarray(x, dtype=np.float32)
    weights = np.asarray(weights, dtype=np.float32)

    # xt[p, kc, r]: longitude-on-partitions view of x, r = (comp, ch, lat)
    xf = np.transpose(x[0], (3, 1, 0, 2)).reshape(NLON, R)
    xt = np.ascontiguousarray(
        xf.reshape(NKC, KC, R).transpose(1, 0, 2)).astype(np.float16)

    # permutation matrices for the PE transposes (must be square 0/1 perms)
    g = np.zeros((PADM, 2, PADM), dtype=np.float16)
    g[:, 0, :] = np.eye(PADM, dtype=np.float16)      # comp0: identity
    for i in range(64):
        g[64 + i, 1, i] = 1    # comp1: sin block -> slot block 0
        g[i, 1, 64 + i] = 1    # comp1: cos block -> slot block 1

    n = np.arange(NLON, dtype=np.float64)
    in_maps = []
    for c in range(NCORES):
        mb = c * MPC
        m = mb + np.arange(MPC, dtype=np.float64)
        ang = 2.0 * np.pi * np.outer(n, m) / NLON
        s = 2.0 * np.pi / NLON
        cosm = np.cos(ang) * s
        sinm = np.sin(ang) * s
        valid = (mb + np.arange(MPC)) < MMAX
        cosm[:, ~valid] = 0.0
        sinm[:, ~valid] = 0.0
        dft = np.zeros((NLON, PADM), dtype=np.float64)
        dft[:, 0:MPC] = cosm
        dft[:, 64:64 + MPC] = sinm
        dftm = np.ascontiguousarray(
            dft.reshape(NKC, KC, PADM).transpose(1, 0, 2)).astype(np.float16)

        take = max(0, min(MPC, MMAX - mb))
        wc = np.zeros((2, MPC, LMAX, NLAT), dtype=np.float32)
        if take:
            wc[:, :take] = weights[:, mb:mb + take]
        # wts[j, p, kc, i*360 + l] = wc[i, j, l, kc*120 + p]
        tmp = wc.transpose(1, 3, 0, 2)                      # (j, k, i, l)
        tmp = tmp.reshape(MPC, LKC, KC, 2, LMAX)
        tmp = tmp.transpose(0, 2, 1, 3, 4)                  # (j, p, kc, i, l)
        wts = np.ascontiguousarray(
            tmp.reshape(MPC, KC, LKC, 2 * LMAX)).astype(np.float16)

        in_maps.append({"xt": xt, "dftm": dftm, "gmat": g, "wts": wts})
    return in_maps


def _assemble(results):
    full = np.empty((1, CH, 2, LMAX, MMAX), dtype=np.complex64)
    for c in range(NCORES):
        mb = c * MPC
        take = max(0, min(MPC, MMAX - mb))
        if not take:
            continue
        o = results[c]["out"].astype(np.float32)  # [46, 64, 2, 360]
        out0 = (o[:, 0:CH, 0, :] + 1j * o[:, CH:64, 0, :]).astype(np.complex64)
        out1 = (-o[:, CH:64, 1, :] + 1j * o[:, 0:CH, 1, :]).astype(np.complex64)
        # (j, ch, l) -> (ch, l, j)
        full[0, :, 0, :, mb:mb + take] = out0.transpose(1, 2, 0)[:, :, :take]
        full[0, :, 1, :, mb:mb + take] = out1.transpose(1, 2, 0)[:, :, :take]
    return full


def _run(x, weights, trace=False):
    if "nc" not in _CACHE:
        _CACHE["nc"] = _build_program()
    nc = _CACHE["nc"]
    in_maps = _prep_in_maps(x, weights)
    res = run_bass_kernel_spmd(nc, in_maps, list(range(NCORES)), trace=trace)
    return _assemble(res.results), res


def kernel(x, weights):
    out, _ = _run(x, weights, trace=False)
    return out



# revision 2
# speedup vs baseline: 1.0507x; 1.0507x over previous
"""Distributed real-vector SHT on 8 Trainium2 NeuronCores — v3.

Full inputs in, full output out. Internally:

  stage 1 (DFT):     channel-parallel. Core c holds 4 of the 32 channels
                     (both vector components) and computes, for ALL 368
                     (padded) azimuthal modes, the longitude DFT
                     y[lat, m] = sum_n x[n, lat] * trig[n, m]  (matmul,
                     lat on PSUM partitions — no transposes needed).
  stage 2 (a2a):     AllToAll redistributes y from channel-sharded to
                     mode-sharded (46 modes per core), chunked over the 3
                     latitude blocks so transport overlaps stage-1 compute.
  stage 3 (Legendre): per mode j, contract over latitude with the
                     quadrature weights. The complex recombination is folded
                     into PSUM accumulation: two matmul passes (w0-pass on
                     yt, w1-pass on the sign-permuted yt2) leave the final
                     out0re/out0im/out1re/out1im planes directly in PSUM.

Host only does layout shuffles, dtype casts, sign folding into constant
matrices, and the final complex packing.
"""

import sys
import numpy as np
from contextlib import ExitStack

sys.path.insert(0, "/opt/trn_rl_repo")

import concourse.bass as bass  # noqa: E402
import concourse.tile as tile  # noqa: E402
from concourse import bacc  # noqa: E402
from concourse import mybir  # noqa: E402
from concourse.bass_utils import run_bass_kernel_spmd  # noqa: E402

NLAT, NLON = 360, 720
LMAX, MMAX = 360, 361
NCORES = 8
MPC = 46            # modes per core (8*46 = 368 >= 361, zero-padded)
M8 = NCORES * MPC   # 368
CH = 32
CPC = CH // NCORES  # 4 channels per core
NQ = 2 * CPC        # 8 (comp, chl) pairs per core
LC = 3              # latitude chunks of 120
LCH = NLAT // LC    # 120
NK = 6              # longitude chunks of 120
KCH = NLON // NK    # 120
ROWB = 2 * 2 * CPC  # 16 = (comp, trig, chl) rows per (j, src)
JROW = MPC * ROWB   # 736 = per-lat row of a y block
WPRE = 8           # weights prefetched during stages 1-2
F16 = mybir.dt.float16
F32 = mybir.dt.float32

_CACHE = {}


def _build_program(reps=1, mode="full"):
    """mode: 'full' = with AllToAll; 'nocc' = skip collective (timing sim
    only — stage 3 then reads the core's own send buffer)."""
    nc = bacc.Bacc("TRN2", target_bir_lowering=False, debug=False,
                   num_devices=NCORES)
    xt = nc.dram_tensor("xt", [NQ, KCH, NK, NLAT], F16,
                        kind="ExternalInput").ap()
    trg = nc.dram_tensor("trg", [KCH, NK, 2, M8], F16,
                         kind="ExternalInput").ap()
    wts = nc.dram_tensor("wts", [MPC, LCH, LC, 2 * LMAX], F16,
                         kind="ExternalInput").ap()
    out = nc.dram_tensor("out", [4 * CH, MPC, LMAX], F16,
                         kind="ExternalOutput").ap()
    y_send = nc.dram_tensor("y_send", [LC, NCORES, LCH, JROW], F16).ap()
    y_recv = nc.dram_tensor("y_recv", [LC, NCORES, LCH, JROW], F16).ap()

    with tile.TileContext(nc) as tc, ExitStack() as ctx:
        const_pool = ctx.enter_context(tc.tile_pool(name="const", bufs=1))
        # weight pool at top level: prefetch during stages 1-2
        w_pool = ctx.enter_context(tc.tile_pool(name="win", bufs=WPRE // 2))

        trg_t = const_pool.tile([KCH, NK, 2, M8], F16, tag="trg")
        nc.scalar.dma_start(trg_t[:, 0:3], trg[:, 0:3])
        nc.scalar.dma_start(trg_t[:, 3:6], trg[:, 3:6])

        for _rep in range(reps):
            _build_body(nc, tc, xt, wts, out, y_send, y_recv, trg_t,
                        w_pool, mode)

    nc.compile()
    return nc


def _build_body(nc, tc, xt, wts, out, y_send, y_recv, trg_t, w_pool, mode):
    w_tiles = {}
    wq = [0]

    def prefetch_w(n):
        while wq[0] < min(MPC, n):
            j = wq[0]
            wt = w_pool.tile([LCH, 2, LC, 2 * LMAX], F16, tag="win",
                             name=f"w{j}")
            nc.scalar.dma_start(wt[:], wts[j:j + 2].transpose([1, 0, 2, 3]))
            w_tiles[j] = wt
            wq[0] += 2

    yrd = y_send if mode == "nocc" else y_recv

    # all pools coexist so receive/scatter overlap stage-1 compute
    with tc.tile_pool(name="xin", bufs=1) as x_pool, \
         tc.tile_pool(name="ys", bufs=2) as ys_pool, \
         tc.tile_pool(name="yr", bufs=2) as yr_pool, \
         tc.tile_pool(name="yt", bufs=LC) as yt_pool, \
         tc.tile_pool(name="yt2", bufs=LC) as yt2_pool, \
         tc.tile_pool(name="dps", bufs=2, space="PSUM") as ps_pool, \
         tc.tile_pool(name="lps", bufs=4, space="PSUM") as lps_pool, \
         tc.tile_pool(name="osb", bufs=2) as o_pool:
        x_t = []
        for q in range(NQ):
            x_t.append(x_pool.tile([KCH, NK, NLAT], F16, tag=f"xin{q}",
                                   name=f"x{q}"))
            nc.sync.dma_start(x_t[q][:], xt[q])

        yt_t, yt2_t = [], []
        for lc in range(LC):
            # ---- stage 1: DFT for latitude chunk lc ----
            ys_t = ys_pool.tile([LCH, NCORES, MPC, 2, 2, CPC], F16,
                                tag="ys", name=f"ys{lc}")
            for q in range(NQ):
                comp, chl = divmod(q, CPC)
                ps = [ps_pool.tile([LCH, M8], F32, tag=f"ps{t}",
                                   name=f"ps{t}_{lc}_{q}") for t in range(2)]
                for kch in range(NK):
                    lhsT = x_t[q][:, kch, lc * LCH:(lc + 1) * LCH]
                    for t in range(2):
                        nc.tensor.matmul(
                            ps[t][:],
                            lhsT=lhsT,
                            rhs=trg_t[:, kch, t, :],
                            start=(kch == 0), stop=(kch == NK - 1),
                        )
                # scatter into ys[:, mb, j, comp, t, chl]
                for t in range(2):
                    dst = ys_t[:, :, :, comp, t, chl]
                    src = ps[t].rearrange("p (mb j) -> p mb j", mb=NCORES)
                    if (q + t) % 2 == 0:
                        nc.vector.tensor_copy(dst, src)
                    else:
                        nc.scalar.copy(dst, src)
            nc.sync.dma_start(
                y_send[lc].transpose([1, 0, 2]),
                ys_t.rearrange("p mb j c t l -> p mb (j c t l)"))
            if mode == "full":
                nc.gpsimd.collective_compute(
                    "AllToAll",
                    mybir.AluOpType.bypass,
                    replica_groups=[list(range(NCORES))],
                    ins=[y_send[lc].opt()],
                    outs=[y_recv[lc].opt()],
                )

            # ---- stage 2b: receive + build yt / yt2 for chunk lc ----
            # yt blocks (b = 2*comp + trig): [a, b, -c, -d]
            # yt2 blocks: [+ (1,1), - (1,0), - (0,1), + (0,0)]
            yr_t = yr_pool.tile([LCH, NCORES, MPC, 2, 2, CPC], F16,
                                tag="yr", name=f"yr{lc}")
            nc.sync.dma_start(
                yr_t.rearrange("p s j c t l -> p s (j c t l)"),
                yrd[lc].transpose([1, 0, 2]))
            yt_t.append(yt_pool.tile([LCH, MPC, 4, CH], F16, tag="yt",
                                     name=f"yt{lc}"))
            yt2_t.append(yt2_pool.tile([LCH, MPC, 4, CH], F16, tag="yt2",
                                       name=f"yt2{lc}"))
            for comp in range(2):
                for t in range(2):
                    src = yr_t[:, :, :, comp, t, :]
                    b = 2 * comp + t
                    dst = yt_t[lc].rearrange(
                        "p j b (s c) -> p s j b c", s=NCORES)[:, :, :, b, :]
                    if t == 0:
                        nc.vector.tensor_copy(dst, src)
                    else:
                        nc.scalar.copy(dst, src)
                    b2 = 3 - b
                    dst2 = yt2_t[lc].rearrange(
                        "p j b (s c) -> p s j b c", s=NCORES)[:, :, :, b2, :]
                    neg = (b == 1 or b == 2)
                    if neg:
                        nc.vector.tensor_scalar_mul(dst2, src, -1.0)
                    else:
                        nc.scalar.copy(dst2, src)
            prefetch_w(WPRE * (lc + 1) // LC)

        # ---- stage 3: Legendre + PSUM-accumulated combine ----
        for j in range(MPC):
            if j % 2 == 0:
                if j in w_tiles:
                    w_pair = w_tiles.pop(j)
                else:
                    w_pair = w_pool.tile([LCH, 2, LC, 2 * LMAX], F16,
                                         tag="win")
                    nc.scalar.dma_start(w_pair[:], wts[j:j + 2].transpose([1, 0, 2, 3]))
            w_t = w_pair[:, j % 2]

            ps3 = lps_pool.tile([4 * CH, LMAX], F32, tag="lps",
                                name=f"lp{j}")
            for lc in range(LC):
                nc.tensor.matmul(
                    ps3[:], lhsT=yt_t[lc][:, j], rhs=w_t[:, lc, 0:LMAX],
                    start=(lc == 0), stop=False,
                )
            for lc in range(LC):
                nc.tensor.matmul(
                    ps3[:], lhsT=yt2_t[lc][:, j],
                    rhs=w_t[:, lc, LMAX:2 * LMAX],
                    start=False, stop=(lc == LC - 1),
                )
            if j % 8 == 0:
                osb = o_pool.tile([4 * CH, 8, LMAX], F16, tag="osb")
            if j % 2 == 0:
                nc.vector.tensor_copy(osb[:, j % 8], ps3[:])
            else:
                nc.scalar.copy(osb[:, j % 8], ps3[:])
            if j % 8 == 7 or j == MPC - 1:
                jb = (j // 8) * 8
                nc.sync.dma_start(
                    out[:, jb:j + 1, :],
                    osb[:, 0:(j % 8) + 1])


def _prep_in_maps(x, weights):
    x = np.asarray(x, dtype=np.float32)
    weights = np.asarray(weights, dtype=np.float32)

    # trig matrix: t=0 cos*s, t=1 -sin*s; zero-padded above MMAX
    s = 2.0 * np.pi / NLON
    n = np.arange(NLON, dtype=np.float64)
    m = np.arange(M8, dtype=np.float64)
    ang = 2.0 * np.pi * np.outer(n, m) / NLON
    trig = np.zeros((NLON, 2, M8), dtype=np.float64)
    trig[:, 0, :] = np.cos(ang) * s
    trig[:, 1, :] = -np.sin(ang) * s
    trig[:, :, MMAX:] = 0.0
    trg = np.ascontiguousarray(
        trig.reshape(NK, KCH, 2, M8).transpose(1, 0, 2, 3)).astype(np.float16)

    in_maps = []
    for c in range(NCORES):
        # xt[q=(comp,chl), p, kc, lat] = (-1)^comp * x[0, 4c+chl, comp,
        #                                            lat, kc*120+p]
        xc = x[0, CPC * c:CPC * (c + 1)]          # (4, 2, 360, 720)
        xq = np.transpose(xc, (1, 0, 3, 2))       # (comp, chl, 720, 360)
        xq = xq.copy()
        xq[1] *= -1.0
        xq = xq.reshape(NQ, NK, KCH, NLAT).transpose(0, 2, 1, 3)
        xtc = np.ascontiguousarray(xq).astype(np.float16)

        mb = c * MPC
        take = max(0, min(MPC, MMAX - mb))
        wc = np.zeros((2, MPC, LMAX, NLAT), dtype=np.float32)
        if take:
            wc[:, :take] = weights[:, mb:mb + take]
        # wts[j, p, kc, i*360 + l] = wc[i, j, l, kc*120 + p]
        tmp = wc.transpose(1, 3, 0, 2)                      # (j, k, i, l)
        tmp = tmp.reshape(MPC, LC, LCH, 2, LMAX)
        tmp = tmp.transpose(0, 2, 1, 3, 4)                  # (j, p, kc, i, l)
        wtc = np.ascontiguousarray(
            tmp.reshape(MPC, LCH, LC, 2 * LMAX)).astype(np.float16)

        in_maps.append({"xt": xtc, "trg": trg, "wts": wtc})
    return in_maps


def _assemble(results):
    full = np.empty((1, CH, 2, LMAX, MMAX), dtype=np.complex64)
    for c in range(NCORES):
        mb = c * MPC
        take = max(0, min(MPC, MMAX - mb))
        if not take:
            continue
        o = results[c]["out"].astype(np.float32)  # [128, 46, 360]
        out0 = (o[0:CH] + 1j * o[CH:2 * CH]).astype(np.complex64)
        out1 = (o[2 * CH:3 * CH] + 1j * o[3 * CH:4 * CH]).astype(np.complex64)
        # (ch, j, l) -> (ch, l, j)
        full[0, :, 0, :, mb:mb + take] = out0.transpose(0, 2, 1)[:, :, :take]
        full[0, :, 1, :, mb:mb + take] = out1.transpose(0, 2, 1)[:, :, :take]
    return full


def _run(x, weights, trace=False):
    if "nc" not in _CACHE:
        _CACHE["nc"] = _build_program()
    nc = _CACHE["nc"]
    in_maps = _prep_in_maps(x, weights)
    res = run_bass_kernel_spmd(nc, in_maps, list(range(NCORES)), trace=trace)
    return _assemble(res.results), res


def kernel(x, weights):
    out, _ = _run(x, weights, trace=False)
    return out


# revision 3
# speedup vs baseline: 1.2085x; 1.1503x over previous
"""Distributed real-vector SHT on 8 Trainium2 NeuronCores — v3.

Full inputs in, full output out. Internally:

  stage 1 (DFT):     channel-parallel. Core c holds 4 of the 32 channels
                     (both vector components) and computes, for ALL 368
                     (padded) azimuthal modes, the longitude DFT
                     y[lat, m] = sum_n x[n, lat] * trig[n, m]  (matmul,
                     lat on PSUM partitions — no transposes needed).
  stage 2 (a2a):     AllToAll redistributes y from channel-sharded to
                     mode-sharded (46 modes per core), chunked over the 3
                     latitude blocks so transport overlaps stage-1 compute.
  stage 3 (Legendre): per mode j, contract over latitude with the
                     quadrature weights. The complex recombination is folded
                     into PSUM accumulation: two matmul passes (w0-pass on
                     yt, w1-pass on the sign-permuted yt2) leave the final
                     out0re/out0im/out1re/out1im planes directly in PSUM.

Host only does layout shuffles, dtype casts, sign folding into constant
matrices, and the final complex packing.
"""

import sys
import numpy as np
from contextlib import ExitStack

sys.path.insert(0, "/opt/trn_rl_repo")

import concourse.bass as bass  # noqa: E402
import concourse.tile as tile  # noqa: E402
from concourse import bacc  # noqa: E402
from concourse import mybir  # noqa: E402
from concourse.bass_utils import run_bass_kernel_spmd  # noqa: E402

NLAT, NLON = 360, 720
LMAX, MMAX = 360, 361
NCORES = 8
MPC = 46            # modes per core (8*46 = 368 >= 361, zero-padded)
M8 = NCORES * MPC   # 368
CH = 32
CPC = CH // NCORES  # 4 channels per core
NQ = 2 * CPC        # 8 (comp, chl) pairs per core
LC = 3              # latitude chunks of 120
LCH = NLAT // LC    # 120
NK = 6              # longitude chunks of 120
KCH = NLON // NK    # 120
ROWB = 2 * 2 * CPC  # 16 = (comp, trig, chl) rows per (j, src)
JROW = MPC * ROWB   # 736 = per-lat row of a y block
WPRE = 6           # weights prefetched during stages 1-2
F16 = mybir.dt.float16
F32 = mybir.dt.float32

_CACHE = {}


def _build_program(reps=1, mode="full"):
    """mode: 'full' = with AllToAll; 'nocc' = skip collective (timing sim
    only — stage 3 then reads the core's own send buffer)."""
    nc = bacc.Bacc("TRN2", target_bir_lowering=False, debug=False,
                   num_devices=NCORES)
    xlo = nc.dram_tensor("xlo", [NQ, KCH, 3, NLAT], F16,
                         kind="ExternalInput").ap()
    xrv = nc.dram_tensor("xrv", [NQ, KCH, 3, NLAT], F16,
                         kind="ExternalInput").ap()
    x360 = nc.dram_tensor("x360", [NQ, 1, NLAT], F16,
                          kind="ExternalInput").ap()
    trg = nc.dram_tensor("trg", [KCH + 1, 7, M8], F16,
                         kind="ExternalInput").ap()
    wts = nc.dram_tensor("wts", [MPC, LCH, LC, 2 * LMAX], F16,
                         kind="ExternalInput").ap()
    out = nc.dram_tensor("out", [4 * CH, MPC, LMAX], F16,
                         kind="ExternalOutput").ap()
    y_send = nc.dram_tensor("y_send", [LC, NCORES, LCH, JROW], F16).ap()
    y_recv = nc.dram_tensor("y_recv", [LC, NCORES, LCH, JROW], F16).ap()

    with tile.TileContext(nc) as tc, ExitStack() as ctx:
        const_pool = ctx.enter_context(tc.tile_pool(name="const", bufs=1))
        # weight pool at top level: prefetch during stages 1-2
        w_pool = ctx.enter_context(tc.tile_pool(name="win", bufs=WPRE // 2))

        trg_t = const_pool.tile([KCH + 1, 7, M8], F16, tag="trg")
        nc.scalar.dma_start(trg_t[:, 0:4], trg[:, 0:4])
        nc.scalar.dma_start(trg_t[:, 4:7], trg[:, 4:7])

        for _rep in range(reps):
            _build_body(nc, tc, xlo, xrv, x360, wts, out, y_send, y_recv,
                        trg_t, w_pool, mode)

    nc.compile()
    return nc


def _build_body(nc, tc, xlo, xrv, x360, wts, out, y_send, y_recv, trg_t, w_pool, mode):
    w_tiles = {}
    wq = [0]

    def prefetch_w(n):
        while wq[0] < min(MPC, n):
            j = wq[0]
            wt = w_pool.tile([LCH, 2, LC, 2 * LMAX], F16, tag="win",
                             name=f"w{j}")
            nc.scalar.dma_start(wt[:], wts[j:j + 2].transpose([1, 0, 2, 3]))
            w_tiles[j] = wt
            wq[0] += 2

    yrd = y_send if mode == "nocc" else y_recv

    # all pools coexist so receive/scatter overlap stage-1 compute
    with tc.tile_pool(name="xin", bufs=2) as x_pool, \
         tc.tile_pool(name="uv", bufs=1) as uv_pool, \
         tc.tile_pool(name="ys", bufs=2) as ys_pool, \
         tc.tile_pool(name="yr", bufs=2) as yr_pool, \
         tc.tile_pool(name="yt", bufs=LC) as yt_pool, \
         tc.tile_pool(name="yt2", bufs=LC) as yt2_pool, \
         tc.tile_pool(name="dps", bufs=2, space="PSUM") as ps_pool, \
         tc.tile_pool(name="lps", bufs=4, space="PSUM") as lps_pool, \
         tc.tile_pool(name="osb", bufs=2) as o_pool:
        u_t, v_t = [], []
        for q in range(NQ):
            xl = x_pool.tile([KCH, 3, NLAT], F16, tag="xl", name=f"xl{q}")
            nc.sync.dma_start(xl[:], xlo[q])
            xr = x_pool.tile([KCH, 3, NLAT], F16, tag="xr", name=f"xr{q}")
            nc.sync.dma_start(xr[:], xrv[q])
            u = uv_pool.tile([KCH + 1, 3, NLAT], F16, tag=f"u{q}",
                             name=f"u{q}")
            v = uv_pool.tile([KCH, 3, NLAT], F16, tag=f"v{q}", name=f"v{q}")
            nc.scalar.dma_start(u[120:121, 2, :], x360[q])
            nc.vector.tensor_add(u[0:KCH, :, :], xl[:], xr[:])
            nc.vector.tensor_sub(v[:], xl[:], xr[:])
            u_t.append(u)
            v_t.append(v)

        yt_t, yt2_t = [], []
        for lc in range(LC):
            # ---- stage 1: DFT for latitude chunk lc ----
            ys_t = ys_pool.tile([LCH, NCORES, MPC, 2, 2, CPC], F16,
                                tag="ys", name=f"ys{lc}")
            for q in range(NQ):
                comp, chl = divmod(q, CPC)
                ps = [ps_pool.tile([LCH, M8], F32, tag=f"ps{t}",
                                   name=f"ps{t}_{lc}_{q}") for t in range(2)]
                sl = slice(lc * LCH, (lc + 1) * LCH)
                for ci in range(3):
                    kk = KCH + 1 if ci == 2 else KCH
                    nc.tensor.matmul(
                        ps[0][:], lhsT=u_t[q][0:kk, ci, sl],
                        rhs=trg_t[0:kk, ci, :],
                        start=(ci == 0), stop=(ci == 2),
                    )
                for ci in range(3):
                    nc.tensor.matmul(
                        ps[1][:], lhsT=v_t[q][0:KCH, ci, sl],
                        rhs=trg_t[0:KCH, 4 + ci, :],
                        start=(ci == 0), stop=(ci == 2),
                    )
                # scatter into ys[:, mb, j, comp, t, chl]
                for t in range(2):
                    dst = ys_t[:, :, :, comp, t, chl]
                    src = ps[t].rearrange("p (mb j) -> p mb j", mb=NCORES)
                    if (q + t) % 2 == 0:
                        nc.vector.tensor_copy(dst, src)
                    else:
                        nc.scalar.copy(dst, src)
            nc.sync.dma_start(
                y_send[lc].transpose([1, 0, 2]),
                ys_t.rearrange("p mb j c t l -> p mb (j c t l)"))
            if mode == "full":
                nc.gpsimd.collective_compute(
                    "AllToAll",
                    mybir.AluOpType.bypass,
                    replica_groups=[list(range(NCORES))],
                    ins=[y_send[lc].opt()],
                    outs=[y_recv[lc].opt()],
                )

            # ---- stage 2b: receive + build yt / yt2 for chunk lc ----
            # yt blocks (b = 2*comp + trig): [a, b, -c, -d]
            # yt2 blocks: [+ (1,1), - (1,0), - (0,1), + (0,0)]
            yr_t = yr_pool.tile([LCH, NCORES, MPC, 2, 2, CPC], F16,
                                tag="yr", name=f"yr{lc}")
            nc.sync.dma_start(
                yr_t.rearrange("p s j c t l -> p s (j c t l)"),
                yrd[lc].transpose([1, 0, 2]))
            yt_t.append(yt_pool.tile([LCH, MPC, 4, CH], F16, tag="yt",
                                     name=f"yt{lc}"))
            yt2_t.append(yt2_pool.tile([LCH, MPC, 4, CH], F16, tag="yt2",
                                       name=f"yt2{lc}"))
            for comp in range(2):
                for t in range(2):
                    src = yr_t[:, :, :, comp, t, :]
                    b = 2 * comp + t
                    dst = yt_t[lc].rearrange(
                        "p j b (s c) -> p s j b c", s=NCORES)[:, :, :, b, :]
                    if t == 0:
                        nc.vector.tensor_copy(dst, src)
                    else:
                        nc.scalar.copy(dst, src)
                    b2 = 3 - b
                    dst2 = yt2_t[lc].rearrange(
                        "p j b (s c) -> p s j b c", s=NCORES)[:, :, :, b2, :]
                    neg = (b == 1 or b == 2)
                    if neg:
                        nc.vector.tensor_scalar_mul(dst2, src, -1.0)
                    else:
                        nc.scalar.copy(dst2, src)
            prefetch_w(WPRE * (lc + 1) // LC)

        # ---- stage 3: Legendre + PSUM-accumulated combine ----
        for j in range(MPC):
            if j % 2 == 0:
                if j in w_tiles:
                    w_pair = w_tiles.pop(j)
                else:
                    w_pair = w_pool.tile([LCH, 2, LC, 2 * LMAX], F16,
                                         tag="win")
                    nc.scalar.dma_start(w_pair[:], wts[j:j + 2].transpose([1, 0, 2, 3]))
            w_t = w_pair[:, j % 2]

            ps3 = lps_pool.tile([4 * CH, LMAX], F32, tag="lps",
                                name=f"lp{j}")
            for lc in range(LC):
                nc.tensor.matmul(
                    ps3[:], lhsT=yt_t[lc][:, j], rhs=w_t[:, lc, 0:LMAX],
                    start=(lc == 0), stop=False,
                )
            for lc in range(LC):
                nc.tensor.matmul(
                    ps3[:], lhsT=yt2_t[lc][:, j],
                    rhs=w_t[:, lc, LMAX:2 * LMAX],
                    start=False, stop=(lc == LC - 1),
                )
            if j % 4 == 0:
                osb = o_pool.tile([4 * CH, 4, LMAX], F16, tag="osb")
            if j % 2 == 0:
                nc.vector.tensor_copy(osb[:, j % 4], ps3[:])
            else:
                nc.scalar.copy(osb[:, j % 4], ps3[:])
            if j % 4 == 3 or j == MPC - 1:
                jb = (j // 4) * 4
                nc.sync.dma_start(
                    out[:, jb:j + 1, :],
                    osb[:, 0:(j % 4) + 1])


def _prep_in_maps(x, weights):
    x = np.asarray(x, dtype=np.float32)
    weights = np.asarray(weights, dtype=np.float32)

    # folded trig matrix [p, 7, m]: chunks 0-2 cos (n=120c+p), 3 = row 360,
    # 4-6 = -sin. Endpoint rows are single (u[0]=x[0], u[360]=x[360]).
    s = 2.0 * np.pi / NLON
    m = np.arange(M8, dtype=np.float64)
    trig = np.zeros((KCH + 1, 7, M8), dtype=np.float64)
    for c in range(3):
        n = (120 * c + np.arange(KCH + 1, dtype=np.float64))[:, None]
        ang = 2.0 * np.pi * n * m[None, :] / NLON
        trig[:, c, :] = np.cos(ang) * s
        trig[0:KCH, 4 + c, :] = (-np.sin(ang) * s)[0:KCH]
    trig[0, 4, :] = 0.0                       # Ms row n=0 kills v[0]
    trig[:, :, MMAX:] = 0.0
    trg = trig.astype(np.float16)

    in_maps = []
    for c in range(NCORES):
        # xt[q=(comp,chl), p, kc, lat] = (-1)^comp * x[0, 4c+chl, comp,
        #                                            lat, kc*120+p]
        xc = x[0, CPC * c:CPC * (c + 1)]          # (4, 2, 360, 720)
        xq = np.transpose(xc, (1, 0, 3, 2))       # (comp, chl, 720, 360)
        xq = xq.copy()
        xq[1] *= -1.0
        xq = xq.reshape(NQ, NLON, NLAT)      # (q, n, lat)
        xloc = np.ascontiguousarray(
            xq[:, 0:360].reshape(NQ, 3, KCH, NLAT).transpose(0, 2, 1, 3)
        ).astype(np.float16)
        xrvc = np.zeros((NQ, KCH, 3, NLAT), dtype=np.float32)
        for c3 in range(3):
            for p in range(KCH):
                nsrc = NLON - 120 * c3 - p
                if nsrc >= NLON:
                    continue                  # (c0, p0) stays zero
                xrvc[:, p, c3, :] = xq[:, nsrc, :]
        xrvc = xrvc.astype(np.float16)
        x360c = np.ascontiguousarray(xq[:, 360:361, :]).astype(np.float16)

        mb = c * MPC
        take = max(0, min(MPC, MMAX - mb))
        wc = np.zeros((2, MPC, LMAX, NLAT), dtype=np.float32)
        if take:
            wc[:, :take] = weights[:, mb:mb + take]
        # wts[j, p, kc, i*360 + l] = wc[i, j, l, kc*120 + p]
        tmp = wc.transpose(1, 3, 0, 2)                      # (j, k, i, l)
        tmp = tmp.reshape(MPC, LC, LCH, 2, LMAX)
        tmp = tmp.transpose(0, 2, 1, 3, 4)                  # (j, p, kc, i, l)
        wtc = np.ascontiguousarray(
            tmp.reshape(MPC, LCH, LC, 2 * LMAX)).astype(np.float16)

        in_maps.append({"xlo": xloc, "xrv": xrvc, "x360": x360c,
                        "trg": trg, "wts": wtc})
    return in_maps


def _assemble(results):
    full = np.empty((1, CH, 2, LMAX, MMAX), dtype=np.complex64)
    for c in range(NCORES):
        mb = c * MPC
        take = max(0, min(MPC, MMAX - mb))
        if not take:
            continue
        o = results[c]["out"].astype(np.float32)  # [128, 46, 360]
        out0 = (o[0:CH] + 1j * o[CH:2 * CH]).astype(np.complex64)
        out1 = (o[2 * CH:3 * CH] + 1j * o[3 * CH:4 * CH]).astype(np.complex64)
        # (ch, j, l) -> (ch, l, j)
        full[0, :, 0, :, mb:mb + take] = out0.transpose(0, 2, 1)[:, :, :take]
        full[0, :, 1, :, mb:mb + take] = out1.transpose(0, 2, 1)[:, :, :take]
    return full


def _run(x, weights, trace=False):
    if "nc" not in _CACHE:
        _CACHE["nc"] = _build_program()
    nc = _CACHE["nc"]
    in_maps = _prep_in_maps(x, weights)
    res = run_bass_kernel_spmd(nc, in_maps, list(range(NCORES)), trace=trace)
    return _assemble(res.results), res


def kernel(x, weights):
    out, _ = _run(x, weights, trace=False)
    return out
